# revision 1
# baseline (speedup 1.0000x reference)
"""Trainium2 Bass kernel for nn_GCM (GNN message passing / context GCN + FM decoder).

Strategy (8 NeuronCores, SPMD single NEFF):
  - Users/items/contexts/batch are range-sharded across cores.
  - Every segment_sum runs as destination-sorted one-hot matmuls accumulated in
    PSUM (gather -> DVE one-hot -> PE matmul), never scatter-add (HW races on
    duplicate indices).
  - Edge lists are partitioned by destination shard on the host (index-only
    preprocessing); gathers use the dma_gather custom SWDGE instruction
    (int16 indices => large tables are split into <=25000-row slices and the
    per-group edge runs are stratified by slice).
  - Full tables (encoded_u/i, ctx_mean, layer1_u/i) are replicated via
    AllGather collectives; layer-2 is computed only at the 4096 batch rows.
  - FM decode runs fully on-chip; biases are fetched with indirect DMA.

The NEFF is identical on all cores; all per-core variation lives in the input
tensors. Chunk/group structure is padded to the max over cores; padded edges
carry one-hot rank -1 (an all-zero one-hot column) so they contribute nothing.
"""

import numpy as np

import concourse.bacc as bacc
import concourse.bass as bass
import concourse.mybir as mybir
import concourse.tile as tile
from concourse import bass_utils
from concourse.library_config import mlp as _mlp_lib

# ---------------- problem constants (hardcoded; must match reference) -------
U, I, IT, C, E, D, B = 50000, 20000, 22000, 100000, 500000, 64, 4096
NCORES = 8
US, ISH, CS, BS = U // NCORES, I // NCORES, C // NCORES, B // NCORES
P = 128
NCHT = 24          # chunks per gather tile (multiple of OHB)
OHB = 8            # chunks per one-hot DVE op
SPLIT = 25000      # gather-table split size (int16 index headroom)
BLOCK = 49         # max dest groups per psum block (<= 56 psum slots)
F32 = mybir.dt.float32
I16 = mybir.dt.int16
I32 = mybir.dt.int32
ALU = mybir.AluOpType

G_U = (US + P - 1) // P      # 49
G_I = (ISH + P - 1) // P     # 20
G_C = (CS + P - 1) // P      # 98
G_B = (BS + P - 1) // P      # 4


def _dloc_cols(n_chunks):
    # one-hot windows of OHB start at arbitrary in-block offsets; pad so any
    # window starting at the last chunk stays in range
    return (n_chunks // OHB + 2) * OHB


def _wrap_idx(v):
    """Pack an int index list (len % 128 == 0) into the [128, n/16] wrapped +
    8x-replicated int16 layout used by dma_gather: idx i -> [16r + i%16, i//16]."""
    n = len(v)
    a = v.reshape(n // 16, 16).T.astype(np.int16)
    return np.tile(a, (8, 1))


class PassPlan:
    """Core-uniform chunk schedule for one segment-sum pass."""

    def __init__(self, name, n_groups, n_sources):
        self.name = name
        self.n_groups = n_groups
        self.n_sources = n_sources
        self.chunk_group = []      # group id per chunk slot
        self.pieces = []           # (src, split, slot_lo, slot_hi, src_chunk_lo)
        self.blocks = []           # (g0, g1, slot_lo, slot_hi)
        self.n_chunks = 0
        self.runs = []             # (g, slot_lo, slot_hi) consecutive same-(g,s,h)
        self.group_runs = {}       # g -> number of runs
        self.group_last_run = {}   # g -> index into runs of its final run

    def finish(self):
        self.n_chunks = len(self.chunk_group)
        for ri, (g, lo, hi) in enumerate(self.runs):
            self.group_runs[g] = self.group_runs.get(g, 0) + 1
            self.group_last_run[g] = ri


def _build_pass(name, n_groups, entries_per_core, splits, block=BLOCK):
    """Build a core-uniform segsum pass.

    entries_per_core: per core a list over sources of (dest_slot, idx) arrays.
      For "dual" passes both sources cover the same logical edges (pass the
      same dest_slot twice); they become independent chunks (segsum linearity).
      Wait -- no: here every source contributes its own chunks.
    splits: per source the split size (or None for unsplit).

    Returns (plan, per_core_arrays) where per_core_arrays[c] holds
      gidx_{name}_{s} : wrapped int16 [128, n_src_chunks*8]
      dloc_{name}     : f32 [128, ceil(n_chunks/8)*8]
    """
    nsrc = len(splits)
    ncor = len(entries_per_core)
    # bucket counts per (group, src, split)
    buckets = {}
    per_core_b = [dict() for _ in range(ncor)]
    for c in range(ncor):
        for s in range(nsrc):
            slot, idx = entries_per_core[c][s]
            sp = splits[s]
            h = idx // sp if sp else np.zeros_like(idx)
            g = slot // P
            key = (g.astype(np.int64) * 64 + s * 8 + h).astype(np.int64)
            order = np.argsort(key, kind="stable")
            ks, slot_s, idx_s, h_s = key[order], slot[order], idx[order], h[order]
            uk, starts = np.unique(ks, return_index=True)
            starts = list(starts) + [len(ks)]
            for j, k in enumerate(uk):
                gg, rem = divmod(int(k), 64)
                ss, hh = divmod(rem, 8)
                lo, hi = starts[j], starts[j + 1]
                per_core_b[c][(gg, ss, hh)] = (
                    slot_s[lo:hi] % P,
                    idx_s[lo:hi] - hh * (splits[s] or 0),
                )
                prev = buckets.get((gg, ss, hh), 0)
                buckets[(gg, ss, hh)] = max(prev, hi - lo)
    # force >=1 chunk per group
    for g in range(n_groups):
        if not any(k[0] == g for k in buckets):
            buckets[(g, 0, 0)] = 1
    # chunk schedule: (block, src, split, group) order
    plan = PassPlan(name, n_groups, nsrc)
    src_chunks = [0] * nsrc
    gidx_parts = [[[] for _ in range(nsrc)] for _ in range(ncor)]
    dloc_parts = [[] for _ in range(ncor)]
    g0 = 0
    while g0 < n_groups:
        g1 = min(g0 + block, n_groups)
        slot_lo = len(plan.chunk_group)
        for s in range(nsrc):
            hs = sorted({k[2] for k in buckets if k[1] == s})
            for h in hs:
                run_lo = len(plan.chunk_group)
                for g in range(g0, g1):
                    n = buckets.get((g, s, h), 0)
                    if n == 0:
                        continue
                    nch = (n + P - 1) // P
                    npad = nch * P
                    plan.runs.append(
                        (g, len(plan.chunk_group), len(plan.chunk_group) + nch))
                    for c in range(ncor):
                        ent = per_core_b[c].get((g, s, h))
                        if ent is None:
                            ranks = np.full(npad, -1, np.int64)
                            idxs = np.zeros(npad, np.int64)
                        else:
                            r, ix = ent
                            pad = npad - len(r)
                            ranks = np.concatenate([r, np.full(pad, -1, np.int64)])
                            idxs = np.concatenate([ix, np.zeros(pad, np.int64)])
                        gidx_parts[c][s].append(idxs)
                        dloc_parts[c].append((len(plan.chunk_group), ranks))
                    plan.chunk_group.extend([g] * nch)
                run_hi = len(plan.chunk_group)
                if run_hi > run_lo:
                    plan.pieces.append((s, h, run_lo, run_hi, src_chunks[s]))
                    src_chunks[s] += run_hi - run_lo
        plan.blocks.append((g0, g1, slot_lo, len(plan.chunk_group)))
        g0 = g1
    plan.finish()
    plan.src_chunks = src_chunks
    # assemble per-core arrays
    ncol = _dloc_cols(plan.n_chunks)
    out = []
    for c in range(ncor):
        d = np.full((plan.n_chunks, P), -1.0, np.float32)
        for slot0, ranks in dloc_parts[c]:
            nch = len(ranks) // P
            d[slot0:slot0 + nch] = ranks.reshape(nch, P).astype(np.float32)
        dfull = np.full((P, ncol), -1.0, np.float32)
        dfull[:, :plan.n_chunks] = d.T
        arrs = {f"dloc_{name}": dfull}
        for s in range(nsrc):
            v = (np.concatenate(gidx_parts[c][s])
                 if gidx_parts[c][s] else np.zeros(0, np.int64))
            need = max(src_chunks[s], 1) * 8
            a = (_wrap_idx(v) if len(v)
                 else np.zeros((P, 8), np.int16))
            if a.shape[1] < need:
                a = np.concatenate(
                    [a, np.zeros((P, need - a.shape[1]), np.int16)], axis=1)
            arrs[f"gidx_{name}_{s}"] = a
        out.append(arrs)
    return plan, out


_tbl_rows = {}  # unused placeholder kept for _build_pass closure simplicity


# ------------------------------------------------------------------------
def _host_prep(inputs):
    """Pure-integer host preprocessing: edge partitioning + pass plans."""
    u2 = np.asarray(inputs["insts2userid"])
    i2 = np.asarray(inputs["insts2itemid"])
    c2 = np.asarray(inputs["insts2contextid"])
    ufm = np.asarray(inputs["user_feature_mat"])
    ifm = np.asarray(inputs["item_feature_mat"])
    cfm = np.asarray(inputs["context_feature_mat"])
    uid = np.asarray(inputs["user_id"])
    iid = np.asarray(inputs["item_id"])
    cid = np.asarray(inputs["context_id"])

    plans = {}
    percore = [dict() for _ in range(NCORES)]

    def add(plan, arrs):
        plans[plan.name] = plan
        for c in range(NCORES):
            percore[c].update(arrs[c])

    # stage0: encoded_u (3 ufeat rows per user; base handled separately)
    ents = []
    for c in range(NCORES):
        sl = np.repeat(np.arange(US), 3)
        ix = ufm[c * US:(c + 1) * US].ravel().astype(np.int64)
        ents.append([(sl, ix)])
    add(*_build_pass("encu", G_U, ents, [None]))

    ents = []
    for c in range(NCORES):
        sl = np.repeat(np.arange(ISH), 3)
        ix = ifm[c * ISH:(c + 1) * ISH].ravel().astype(np.int64)
        ents.append([(sl, ix)])
    add(*_build_pass("enci", G_I, ents, [None]))

    # stage0: ctx_mean: src0 = 3 cfeat rows, src1 = 1 item row per context
    ents = []
    for c in range(NCORES):
        sh = cfm[c * CS:(c + 1) * CS]
        sl0 = np.repeat(np.arange(CS), 3)
        ix0 = sh[:, :3].ravel().astype(np.int64)
        sl1 = np.arange(CS)
        ix1 = sh[:, 3].astype(np.int64)
        ents.append([(sl0, ix0), (sl1, ix1)])
    add(*_build_pass("ctx", G_C, ents, [None, None]))

    # layer1 user pass: dest=user shard; sources: enc_i rows, ctx rows
    ents = []
    ucore = u2 // US
    for c in range(NCORES):
        m = ucore == c
        sl = (u2[m] % US).astype(np.int64)
        ents.append([(sl, i2[m].astype(np.int64)), (sl, c2[m].astype(np.int64))])
    add(*_build_pass("l1u", G_U, ents, [None, SPLIT]))

    ents = []
    icore = i2 // ISH
    for c in range(NCORES):
        m = icore == c
        sl = (i2[m] % ISH).astype(np.int64)
        ents.append([(sl, u2[m].astype(np.int64)), (sl, c2[m].astype(np.int64))])
    add(*_build_pass("l1i", G_I, ents, [SPLIT, SPLIT]))

    # CSRs for batch-restricted layer2
    uord = np.argsort(u2, kind="stable")
    ustart = np.searchsorted(u2[uord], np.arange(U + 1))
    iord = np.argsort(i2, kind="stable")
    istart = np.searchsorted(i2[iord], np.arange(I + 1))

    ents_u, ents_i = [], []
    for c in range(NCORES):
        bu = uid[c * BS:(c + 1) * BS]
        sl, ia, ca = [], [], []
        for b, u in enumerate(bu):
            ee = uord[ustart[u]:ustart[u + 1]]
            sl.append(np.full(len(ee), b, np.int64))
            ia.append(i2[ee].astype(np.int64))
            ca.append(c2[ee].astype(np.int64))
        sl = np.concatenate(sl) if sl else np.zeros(0, np.int64)
        ia = np.concatenate(ia) if ia else np.zeros(0, np.int64)
        ca = np.concatenate(ca) if ca else np.zeros(0, np.int64)
        ents_u.append([(sl, ia), (sl, ca)])

        bi = iid[c * BS:(c + 1) * BS]
        sl, ua, ca = [], [], []
        for b, it in enumerate(bi):
            ee = iord[istart[it]:istart[it + 1]]
            sl.append(np.full(len(ee), b, np.int64))
            ua.append(u2[ee].astype(np.int64))
            ca.append(c2[ee].astype(np.int64))
        sl = np.concatenate(sl) if sl else np.zeros(0, np.int64)
        ua = np.concatenate(ua) if ua else np.zeros(0, np.int64)
        ca = np.concatenate(ca) if ca else np.zeros(0, np.int64)
        ents_i.append([(sl, ua), (sl, ca)])
    add(*_build_pass("l2u", G_B, ents_u, [None, SPLIT]))
    add(*_build_pass("l2i", G_B, ents_i, [SPLIT, SPLIT]))

    # decode index arrays (512 per core, padded to 512)
    for c in range(NCORES):
        bu = uid[c * BS:(c + 1) * BS].astype(np.int64)
        bi = iid[c * BS:(c + 1) * BS].astype(np.int64)
        bc = cid[c * BS:(c + 1) * BS].astype(np.int64)
        cf = cfm[bc]
        pc = percore[c]
        m0 = (bu < SPLIT)
        pc["d_uh0"] = _wrap_idx(np.where(m0, bu, 0))
        pc["d_uh1"] = _wrap_idx(np.where(~m0, bu - SPLIT, 0))
        pc["d_umask0"] = m0.astype(np.float32).reshape(G_B, P).T.copy()
        pc["d_i"] = _wrap_idx(bi)
        for k in range(3):
            pc[f"d_cf{k}"] = _wrap_idx(cf[:, k].astype(np.int64))
        pc["d_cit"] = _wrap_idx(cf[:, 3].astype(np.int64))
        pc["d_ubias"] = bu.astype(np.int32).reshape(G_B, P).T.copy()
        pc["d_ibias"] = bi.astype(np.int32).reshape(G_B, P).T.copy()

    # per-core float inputs (staging/padding only, no arithmetic)
    ue = np.asarray(inputs["user_emb"], np.float32)
    ie = np.asarray(inputs["item_emb"], np.float32)
    for c in range(NCORES):
        pc = percore[c]
        ub = np.zeros((G_U * P, D), np.float32)
        ub[:US] = ue[c * US:(c + 1) * US]
        pc["ue_base"] = ub
        ib = np.zeros((G_I * P, D), np.float32)
        ib[:ISH] = ie[c * ISH:(c + 1) * ISH]
        pc["ie_base"] = ib
        pc["t_ufeat"] = np.asarray(inputs["u_feat_emb"], np.float32)
        pc["t_ifeat"] = np.asarray(inputs["i_feat_emb"], np.float32)
        pc["t_cfeat"] = np.asarray(inputs["c_feat_emb"], np.float32)
        pc["t_item"] = ie
        pc["user_bias"] = np.asarray(inputs["user_bias"], np.float32)
        pc["item_bias"] = np.asarray(inputs["item_bias"], np.float32)
        pc["gbias"] = np.broadcast_to(
            np.asarray(inputs["global_bias"], np.float32).reshape(1, 1), (P, 1)
        ).copy()
    return plans, percore


# ------------------------------------------------------------------------
def _emit_segsum(nc, pools, plan, src_aps, gidx_dram, dloc_dram, write_group,
                 acc):
    """Emit one segsum pass.

    Per (group, source, split) run: one short PSUM chain in its own bank tile,
    then DVE-accumulated into the SBUF ``acc`` tile ([128, n_groups*64]).
    ``write_group(g, acc_slice)`` fires after the group's final run.
    (matmul start=True clears the whole PSUM bank, so chains may never
    interleave within a bank.)
    """
    gpool, ohpool, pspool, idxpool = (pools["g"], pools["oh"], pools["ps"],
                                      pools["idx"])
    dloc_t = pools["dloc"].tile([P, dloc_dram.shape[1]], F32, tag="dloc",
                                name="dloc")
    nc.sync.dma_start(out=dloc_t[:], in_=dloc_dram[:, :])
    iota_t = pools["iota"]

    seen_runs = {}          # g -> runs completed so far
    open_ps = {}            # run idx -> psum tile
    run_at = 0              # next run whose slots we have not fully consumed
    for (g0, g1, slot_lo, slot_hi) in plan.blocks:
        for ts in range(slot_lo, slot_hi, NCHT):
            te = min(ts + NCHT, slot_hi)
            gt = gpool.tile([P, NCHT, D], F32, tag="gA", name="gt")
            for (spc, h, lo, hi, sclo) in plan.pieces:
                a, b = max(lo, ts), min(hi, te)
                if a >= b:
                    continue
                ca = sclo + (a - lo)
                nidx = (b - a) * P
                it = idxpool.tile([P, NCHT * 8], I16, tag="gi", name="gi")
                nc.sync.dma_start(
                    out=it[:, :(b - a) * 8],
                    in_=gidx_dram[spc][:, ca * 8:(ca + (b - a)) * 8],
                )
                nc.gpsimd.dma_gather(
                    gt[:, a - ts:b - ts, :], src_aps[spc][h],
                    it[:, :(b - a) * 8], nidx, nidx, D, single_packet=False,
                )
            ohs = []
            for ob in range(ts, te, OHB):
                oh = ohpool.tile([P, OHB * P], F32, tag="oh", name="oh")
                nc.vector.tensor_tensor(
                    out=oh[:].rearrange("p (c j) -> p c j", j=P),
                    in0=iota_t[:, :].unsqueeze(1).to_broadcast([P, OHB, P]),
                    in1=dloc_t[:, ob:ob + OHB].unsqueeze(2).to_broadcast(
                        [P, OHB, P]),
                    op=ALU.is_equal,
                )
                ohs.append(oh)
            for cslot in range(ts, te):
                while plan.runs[run_at][2] <= cslot:
                    run_at += 1
                g, rlo, rhi = plan.runs[run_at]
                if cslot == rlo:
                    open_ps[run_at] = pspool.tile([P, D], F32, tag="ps",
                                                  space="PSUM", name="ps")
                ps = open_ps[run_at]
                oh = ohs[(cslot - ts) // OHB]
                k = (cslot - ts) % OHB
                nc.tensor.matmul(
                    out=ps[:],
                    lhsT=oh[:, k * P:(k + 1) * P],
                    rhs=gt[:, cslot - ts, :],
                    start=(cslot == rlo),
                    stop=(cslot == rhi - 1),
                )
                if cslot == rhi - 1:
                    del open_ps[run_at]
                    nseen = seen_runs.get(g, 0)
                    accsl = acc[:, g * D:(g + 1) * D]
                    if nseen == 0:
                        nc.vector.tensor_copy(out=accsl, in_=ps[:])
                    else:
                        nc.vector.tensor_tensor(out=accsl, in0=accsl,
                                                in1=ps[:], op=ALU.add)
                    seen_runs[g] = nseen + 1
                    if run_at == plan.group_last_run[g]:
                        write_group(g, accsl)


def build_kernel(plans, debug=False, stage=4):
    nc = bacc.Bacc("TRN2", target_bir_lowering=False, num_devices=NCORES)

    # ---- inputs
    def inp(name, shape, dt=F32):
        return nc.dram_tensor(name, shape, dt, kind="ExternalInput")

    t_ufeat = inp("t_ufeat", [4000, D])
    t_ifeat = inp("t_ifeat", [4000, D])
    t_cfeat = inp("t_cfeat", [1000, D])
    t_item = inp("t_item", [IT, D])
    ue_base = inp("ue_base", [G_U * P, D])
    ie_base = inp("ie_base", [G_I * P, D])
    user_bias = inp("user_bias", [U, 1])
    item_bias = inp("item_bias", [IT, 1])
    gbias = inp("gbias", [P, 1])

    gidx_in, dloc_in = {}, {}
    for name, plan in plans.items():
        dloc_in[name] = inp(f"dloc_{name}", [P, _dloc_cols(plan.n_chunks)])
        gidx_in[name] = [
            inp(f"gidx_{name}_{s}", [P, max(plan.src_chunks[s], 1) * 8], I16)
            for s in range(plan.n_sources)
        ]
    d_in = {}
    for nm in ("d_uh0", "d_uh1", "d_i", "d_cf0", "d_cf1", "d_cf2", "d_cit"):
        d_in[nm] = inp(nm, [P, BS // 16], I16)
    d_umask0 = inp("d_umask0", [P, G_B])
    d_ubias = inp("d_ubias", [P, G_B], I32)
    d_ibias = inp("d_ibias", [P, G_B], I32)

    out_b = nc.dram_tensor("out_b", [BS, 1], F32, kind="ExternalOutput")

    # ---- internal DRAM
    S_encu = nc.dram_tensor("S_encu", [US, D], F32)
    S_enci = nc.dram_tensor("S_enci", [ISH, D], F32)
    S_ctx = nc.dram_tensor("S_ctx", [CS, D], F32)
    S_l1u = nc.dram_tensor("S_l1u", [US, D], F32)
    S_l1i = nc.dram_tensor("S_l1i", [ISH, D], F32)
    T_encu = nc.dram_tensor("T_encu", [U, D], F32, addr_space="Shared")
    T_enci = nc.dram_tensor("T_enci", [I, D], F32, addr_space="Shared")
    T_ctx = nc.dram_tensor("T_ctx", [C, D], F32, addr_space="Shared")
    T_l1u = nc.dram_tensor("T_l1u", [U, D], F32, addr_space="Shared")
    T_l1i = nc.dram_tensor("T_l1i", [I, D], F32, addr_space="Shared")

    dbg_outs = {}
    if debug:
        for nm, t in (("T_encu", T_encu), ("T_enci", T_enci), ("T_ctx", T_ctx),
                      ("T_l1u", T_l1u), ("T_l1i", T_l1i)):
            dbg_outs[nm] = nc.dram_tensor(f"dbg_{nm}", list(t.shape), F32,
                                          kind="ExternalOutput")

    iota_np = np.tile(np.arange(P, dtype=np.float32), (P, 1))
    iota_dram = nc.inline_tensor(iota_np, name="iota128")

    RG = [list(range(NCORES))]

    with tile.TileContext(nc) as tc:
        with (
            tc.tile_pool(name="const", bufs=1) as constp,
            tc.tile_pool(name="g", bufs=4) as gpool,
            tc.tile_pool(name="oh", bufs=6) as ohpool,
            tc.tile_pool(name="idx", bufs=6) as idxpool,
            tc.tile_pool(name="dloc", bufs=2) as dlocp,
            tc.tile_pool(name="base", bufs=2) as basep,
            tc.tile_pool(name="acc", bufs=2) as accp,
            tc.tile_pool(name="tmp", bufs=6) as tmpp,
            tc.tile_pool(name="l2", bufs=1) as l2p,
            tc.tile_pool(name="dec", bufs=1) as decp,
            tc.tile_pool(name="ps", bufs=7, space="PSUM") as pspool,
        ):
            nc.gpsimd.load_library(_mlp_lib)
            iota_t = constp.tile([P, P], F32)
            nc.sync.dma_start(out=iota_t[:], in_=iota_dram[:, :])
            pools = dict(g=gpool, oh=ohpool, ps=pspool, idx=idxpool,
                         dloc=dlocp, iota=iota_t)

            def seg(name, src_aps, write_group, acc=None):
                if acc is None:
                    acc = accp.tile([P, plans[name].n_groups * D], F32,
                                    tag="acc", name="acc")
                _emit_segsum(nc, pools, plans[name], src_aps,
                             gidx_in[name], dloc_in[name], write_group, acc)

            def store_shard(S, g, sl, scale=None, base_t=None, acc=None):
                """psum slice -> (scale/base) -> DRAM shard rows."""
                rows = min(P, S.shape[0] - g * P)
                t = tmpp.tile([P, D], F32, tag="cp", name="cp")
                if scale is not None:
                    nc.vector.tensor_scalar(out=t[:], in0=sl, scalar1=scale,
                                            scalar2=None, op0=ALU.mult)
                else:
                    nc.vector.tensor_copy(out=t[:], in_=sl)
                if base_t is not None:
                    nc.vector.tensor_tensor(
                        out=t[:], in0=t[:],
                        in1=base_t[:, g * D:(g + 1) * D], op=ALU.add)
                nc.sync.dma_start(out=S[g * P:g * P + rows, :], in_=t[:rows, :])

            # ---------- stage0: ctx_mean first (largest AllGather) ----------
            seg("ctx",
                [[t_cfeat[:, :]], [t_item[:, :]]],
                lambda g, sl: store_shard(S_ctx, g, sl, scale=0.25))
            nc.gpsimd.collective_compute(
                "AllGather", ALU.bypass, RG,
                ins=[S_ctx[:, :].opt()], outs=[T_ctx[:, :].opt()])

            # enc_i
            ib_t = basep.tile([P, G_I * D], F32, tag="base")
            nc.sync.dma_start(
                out=ib_t[:].rearrange("p (g d) -> p g d", d=D),
                in_=ie_base[:, :].rearrange("(g p) d -> p g d", p=P))
            nc.vector.tensor_scalar(out=ib_t[:], in0=ib_t[:], scalar1=0.25,
                                    scalar2=None, op0=ALU.mult)
            seg("enci", [[t_ifeat[:, :]]],
                lambda g, sl: store_shard(S_enci, g, sl, 0.25, ib_t))
            nc.gpsimd.collective_compute(
                "AllGather", ALU.bypass, RG,
                ins=[S_enci[:, :].opt()], outs=[T_enci[:, :].opt()])

            # enc_u
            ub_t = basep.tile([P, G_U * D], F32, tag="base")
            nc.sync.dma_start(
                out=ub_t[:].rearrange("p (g d) -> p g d", d=D),
                in_=ue_base[:, :].rearrange("(g p) d -> p g d", p=P))
            nc.vector.tensor_scalar(out=ub_t[:], in0=ub_t[:], scalar1=0.25,
                                    scalar2=None, op0=ALU.mult)
            seg("encu", [[t_ufeat[:, :]]],
                lambda g, sl: store_shard(S_encu, g, sl, 0.25, ub_t))
            nc.gpsimd.collective_compute(
                "AllGather", ALU.bypass, RG,
                ins=[S_encu[:, :].opt()], outs=[T_encu[:, :].opt()])

            # ---------- layer 1 ----------
            ctx_splits = [T_ctx[h * SPLIT:(h + 1) * SPLIT, :] for h in range(4)]
            if stage >= 2:
                seg("l1u", [[T_enci[:, :]], ctx_splits],
                    lambda g, sl: store_shard(S_l1u, g, sl))
                nc.gpsimd.collective_compute(
                    "AllGather", ALU.bypass, RG,
                    ins=[S_l1u[:, :].opt()], outs=[T_l1u[:, :].opt()])

                encu_splits = [T_encu[h * SPLIT:(h + 1) * SPLIT, :]
                               for h in range(2)]
                seg("l1i", [encu_splits, ctx_splits],
                    lambda g, sl: store_shard(S_l1i, g, sl))
                nc.gpsimd.collective_compute(
                    "AllGather", ALU.bypass, RG,
                    ins=[S_l1i[:, :].opt()], outs=[T_l1i[:, :].opt()])

            # ---------- layer 2 (batch-restricted, stays in SBUF) ----------
            if stage >= 3:
                l1u_splits = [T_l1u[h * SPLIT:(h + 1) * SPLIT, :]
                              for h in range(2)]
                acc_l2i = l2p.tile([P, G_B * D], F32, tag="l2i")
                seg("l2i", [l1u_splits, ctx_splits],
                    lambda g, sl: None, acc=acc_l2i)
                acc_l2u = l2p.tile([P, G_B * D], F32, tag="l2u")
                seg("l2u", [[T_l1i[:, :]], ctx_splits],
                    lambda g, sl: None, acc=acc_l2u)

            # ---------- decode ----------
            if stage >= 4:
                def dgather(table_ap, idx_dram, tag):
                    it = idxpool.tile([P, BS // 16], I16, tag="gi", name="gi")
                    nc.sync.dma_start(out=it[:], in_=idx_dram[:, :])
                    t = decp.tile([P, G_B, D], F32, tag=tag, name=tag)
                    nc.gpsimd.dma_gather(t[:], table_ap, it[:], BS, BS, D)
                    return t

                def tt(out, a, b, op):
                    nc.vector.tensor_tensor(out=out, in0=a, in1=b, op=op)

                m0 = decp.tile([P, G_B], F32, tag="m0")
                nc.sync.dma_start(out=m0[:], in_=d_umask0[:, :])
                m0b = m0[:, :].unsqueeze(2).to_broadcast([P, G_B, D])

                # fin_u = (enc_u + l1u + l2u) / 3 at batch users
                eu0 = dgather(T_encu[0:SPLIT, :], d_in["d_uh0"], "eu0")
                eu1 = dgather(T_encu[SPLIT:U, :], d_in["d_uh1"], "eu1")
                lu0 = dgather(T_l1u[0:SPLIT, :], d_in["d_uh0"], "lu0")
                lu1 = dgather(T_l1u[SPLIT:U, :], d_in["d_uh1"], "lu1")
                tt(eu0[:], eu0[:], lu0[:], ALU.add)        # h0 = enc+l1 (half 0)
                tt(eu1[:], eu1[:], lu1[:], ALU.add)        # h1 = enc+l1 (half 1)
                tt(eu0[:], eu0[:], eu1[:], ALU.subtract)   # h0 - h1
                tt(eu0[:], eu0[:], m0b, ALU.mult)          # (h0-h1)*m0
                fin_u = decp.tile([P, G_B, D], F32, tag="finu")
                tt(fin_u[:], eu0[:], eu1[:], ALU.add)      # h1 + (h0-h1)*m0
                l2u_v = acc_l2u[:].rearrange("p (g d) -> p g d", d=D)
                tt(fin_u[:], fin_u[:], l2u_v, ALU.add)
                nc.vector.tensor_scalar(out=fin_u[:], in0=fin_u[:],
                                        scalar1=1.0 / 3.0, scalar2=None,
                                        op0=ALU.mult)

                # fin_i
                ei = dgather(T_enci[:, :], d_in["d_i"], "ei")
                li = dgather(T_l1i[:, :], d_in["d_i"], "li")
                fin_i = decp.tile([P, G_B, D], F32, tag="fini")
                tt(fin_i[:], ei[:], li[:], ALU.add)
                l2i_v = acc_l2i[:].rearrange("p (g d) -> p g d", d=D)
                tt(fin_i[:], fin_i[:], l2i_v, ALU.add)
                nc.vector.tensor_scalar(out=fin_i[:], in0=fin_i[:],
                                        scalar1=1.0 / 3.0, scalar2=None,
                                        op0=ALU.mult)

                # ctx field embeddings
                cf = [dgather(t_cfeat[:, :], d_in[f"d_cf{k}"], f"cf{k}")
                      for k in range(3)]
                cit = dgather(t_item[:, :], d_in["d_cit"], "cit")

                rows = [fin_u, fin_i, cf[0], cf[1], cf[2], cit]
                S = decp.tile([P, G_B, D], F32, tag="S")
                tt(S[:], rows[0][:], rows[1][:], ALU.add)
                for r in rows[2:]:
                    tt(S[:], S[:], r[:], ALU.add)
                SS = decp.tile([P, G_B, D], F32, tag="SS")
                tt(SS[:], S[:], S[:], ALU.mult)
                Q = decp.tile([P, G_B, D], F32, tag="Q")
                tt(Q[:], rows[0][:], rows[0][:], ALU.mult)
                sq = decp.tile([P, G_B, D], F32, tag="sq")
                for r in rows[1:]:
                    tt(sq[:], r[:], r[:], ALU.mult)
                    tt(Q[:], Q[:], sq[:], ALU.add)
                tt(SS[:], SS[:], Q[:], ALU.subtract)
                red = decp.tile([P, G_B], F32, tag="red")
                nc.vector.tensor_reduce(out=red[:].unsqueeze(2), in_=SS[:],
                                        axis=mybir.AxisListType.X, op=ALU.add)
                nc.vector.tensor_scalar(out=red[:], in0=red[:], scalar1=0.5,
                                        scalar2=None, op0=ALU.mult)

                # biases
                ub_i = decp.tile([P, G_B], I32, tag="ubi")
                ib_i = decp.tile([P, G_B], I32, tag="ibi")
                nc.sync.dma_start(out=ub_i[:], in_=d_ubias[:, :])
                nc.sync.dma_start(out=ib_i[:], in_=d_ibias[:, :])
                bu = decp.tile([P, G_B], F32, tag="bu")
                bi = decp.tile([P, G_B], F32, tag="bi")
                for j in range(G_B):
                    nc.gpsimd.indirect_dma_start(
                        out=bu[:, j:j + 1], out_offset=None, in_=user_bias[:, :],
                        in_offset=bass.IndirectOffsetOnAxis(ap=ub_i[:, j:j + 1],
                                                            axis=0))
                    nc.gpsimd.indirect_dma_start(
                        out=bi[:, j:j + 1], out_offset=None, in_=item_bias[:, :],
                        in_offset=bass.IndirectOffsetOnAxis(ap=ib_i[:, j:j + 1],
                                                            axis=0))
                gb_t = decp.tile([P, 1], F32, tag="gb")
                nc.sync.dma_start(out=gb_t[:], in_=gbias[:, :])
                tt(red[:], red[:], bu[:], ALU.add)
                tt(red[:], red[:], bi[:], ALU.add)
                nc.vector.tensor_scalar(out=red[:], in0=red[:],
                                        scalar1=gb_t[:, :1], scalar2=None,
                                        op0=ALU.add)
                nc.sync.dma_start(
                    out=out_b[:, :].rearrange("(g p) d -> p g d", p=P),
                    in_=red[:].unsqueeze(2))

            if debug:
                for nm, t in (("T_encu", T_encu), ("T_enci", T_enci),
                              ("T_ctx", T_ctx), ("T_l1u", T_l1u),
                              ("T_l1i", T_l1i)):
                    o = dbg_outs[nm]
                    flat_o = o[:, :].rearrange("a d -> (a d)").rearrange(
                        "(p x) -> p x", p=P)
                    flat_t = t[:, :].rearrange("a d -> (a d)").rearrange(
                        "(p x) -> p x", p=P)
                    nc.sync.dma_start(out=flat_o, in_=flat_t)

    nc.compile()
    return nc


def kernel(**inputs):
    plans, percore = _host_prep(inputs)
    nc = build_kernel(plans)
    res = bass_utils.run_bass_kernel_spmd(
        nc, percore, core_ids=list(range(NCORES)))
    out = np.concatenate([res.results[c]["out_b"].reshape(-1)
                          for c in range(NCORES)])
    return out.astype(np.float32)

